# revision 27
# baseline (speedup 1.0000x reference)
"""Half-tile-binned gaussian-splat compositing kernel for 8 TRN2 NeuronCores.

Strategy v2 (histogram binning, fp16 pipeline):
  Host (numpy, exact f32 replication of the reference's per-gaussian math):
    - project gaussians, build inverse 2x2 covs, frustum mask, per-tile
      bounding-circle mask (reference semantics), global depth sort.
    - split each 16x16 tile into top/bottom 16x8 half-tiles; a gaussian is
      kept for a half-tile iff it passes the reference's tile-level test AND
      its bcircle (radius 4r > reference's 3r, so dropped tails have
      G <= e^-8) touches the half-tile box. 512 half-tiles are LPT-balanced
      into 16 streams (8 cores x 2 phases, exactly 32 half-tiles each,
      max ~340 slots incl. one dummy reset slot per half-tile).
    - per slot: 6 quadratic-form coefficients in half-tile-local coords such
      that arg = coef . [1,X,Y,X^2,XY,Y^2] = -0.5*quad + log(alpha); coef is
      split hi+lo fp16 and stacked as 12 rows so ONE K=12 fp16 matmul per
      chunk computes hi+lo exactly; colors are packed [128, 3, 96] fp16 so
      every DMA is fully contiguous (no strided descriptors).
  Device (per core, SPMD over 8; partition dim = 128 pixels of one 16x8
  half-tile, free dim = slot stream, phase-major, 3 chunks of 128 slots):
    - PE:  arg[pix, slot] = mono12^T @ coef12  (fp16, fp32 PSUM)
    - ACT: wd = exp(arg - CEXP + 14*ln2)   -> fp16, 2^14-prescaled so all
           fp16 values stay in normal range (dummy slots hit exp = 2^14)
    - POOL:om = 1 - (e^C/2^14)*wd          (= 1-w; dummy slots -> -2980)
    - DVE: scan_s = max(om_s * scan_{s-1}, wd_s) = 2^14 * transmittance,
           fp32 internal state, fp16 out; resets to 2^14 at dummy slots
    - DVE: weight = Tprev * (e^C/2^28) * wd  (= T*w; the reference's
           T>1e-4 gate is dropped: post-threshold contributions sum to
           <= 1e-4 absolute, far under tolerance; dummy-slot weights are
           killed by their zero color rows)
    - PE:  transpose weight 128-chunks (fp16 identity), ACT/DVE/POOL copy
           to SBUF fp16, accumulate out[pix, 96] += weightT^T @ colors
  Output [128, 2*96] fp32 stored contiguously per phase (ph0 overlaps ph1
  compute); host rearranges into the [256,256,3] image.
"""

import numpy as np

N = 1024
H = 256
W = 256
TILE = 16
FX = 300.0
FY = 300.0
CX = 128.0
CY = 128.0
NEAR = 0.1
FAR = 100.0
FRUSTUM_R = 1.0
TILE_CULL_R = 3.0
HALF_CULL_R = 4.0            # half-tile bcircle radius (dropped tails: G<=e^-8)
N_TH = H // TILE
N_TW = W // TILE
N_HH = 2 * N_TH              # 32 rows of 16x8 half-tiles
N_CORES = 8
N_STREAMS = 2 * N_CORES
HT_CAP = 32                  # half-tiles per stream (colm width 96 = 3*32)
SP2 = 384                    # padded stream length (3 transpose chunks of 128)
S2A = {0: 376, 1: 332}       # active slot columns per phase (ph1 lighter: it
                             # ends the kernel, ph0's extra work is overlapped)
PH_OFF = 45                  # LPT load bias applied to ph1 streams
DUMMY_ARG = float(np.log(np.float64(16384.0)))  # exp(dummy) = 2^14 (scan reset)

f = np.float32


def _sigmoid(x):
    return (1.0 / (1.0 + np.exp(-x.astype(np.float64)))).astype(f)


def _host_precompute(mean, qvec, log_svec, color, alpha, c2w):
    """Replicates reference per-gaussian math in f32, bins per half-tile.

    Returns (cm [8,12,128+2*SP2] fp16, colm [8,128,2*288] fp16,
             stream_map: list of 16 lists of (hr, tc))."""
    mean = np.asarray(mean, f)
    qvec = np.asarray(qvec, f)
    log_svec = np.asarray(log_svec, f)
    color = np.asarray(color, f)
    alpha = np.asarray(alpha, f)
    c2w = np.asarray(c2w, f)

    svec = np.exp(log_svec).astype(f)
    a = _sigmoid(alpha)
    Rcw = c2w[:, :3]
    t = c2w[:, 3]
    mean_cam = ((mean - t) @ Rcw).astype(f)
    depth = mean_cam[:, 2]
    zc = np.maximum(depth, f(1e-6))
    inv_z = (f(1.0) / zc).astype(f)
    x, y = mean_cam[:, 0], mean_cam[:, 1]
    mx, my = (x * inv_z).astype(f), (y * inv_z).astype(f)

    q = (qvec / np.linalg.norm(qvec, axis=-1, keepdims=True)).astype(f)
    qw, qx, qy, qz = q[:, 0], q[:, 1], q[:, 2], q[:, 3]
    r0 = np.stack([1 - 2 * (qy * qy + qz * qz), 2 * (qx * qy - qw * qz), 2 * (qx * qz + qw * qy)], -1)
    r1 = np.stack([2 * (qx * qy + qw * qz), 1 - 2 * (qx * qx + qz * qz), 2 * (qy * qz - qw * qx)], -1)
    r2m = np.stack([2 * (qx * qz - qw * qy), 2 * (qy * qz + qw * qx), 1 - 2 * (qx * qx + qy * qy)], -1)
    Rq = np.stack([r0, r1, r2m], axis=1).astype(f)
    zero = np.zeros_like(inv_z)
    J = np.stack([np.stack([inv_z, zero, (-x * inv_z * inv_z).astype(f)], -1),
                  np.stack([zero, inv_z, (-y * inv_z * inv_z).astype(f)], -1)], axis=1).astype(f)
    cov3d = np.einsum('nij,nj,nkj->nik', Rq, (svec * svec).astype(f), Rq).astype(f)
    JW = np.einsum('nij,kj->nik', J, Rcw).astype(f)
    cov = np.einsum('nij,njk,nlk->nil', JW, cov3d, JW).astype(f)
    cov = ((cov + np.swapaxes(cov, -1, -2)) / 2.0).astype(f)
    c00, c01, c11 = cov[:, 0, 0], cov[:, 0, 1], cov[:, 1, 1]
    m = ((c00 + c11) / 2.0).astype(f)
    det = (c00 * c11 - c01 * c01).astype(f)
    radius = np.sqrt(m + np.sqrt(np.clip((m * m - det).astype(f), 0.0, None))).astype(f)

    r3d = (f(FRUSTUM_R) * np.max(svec, axis=-1)).astype(f)
    half_w = f(W / 2.0) / f(FX)
    half_h = f(H / 2.0) / f(FY)
    marg = (r3d * inv_z).astype(f)
    maskf = ((depth > f(NEAR)) & (depth < f(FAR)) &
             (np.abs(mx) < half_w + marg) & (np.abs(my) < half_h + marg))

    psx, psy = f(1.0) / f(FX), f(1.0) / f(FY)
    tlx, tly = f(-CX) / f(FX), f(-CY) / f(FY)
    tx0 = (tlx + np.arange(N_TW, dtype=f) * f(TILE) * psx).astype(f)
    tx1 = (tx0 + f(TILE) * psx).astype(f)
    ty0 = (tly + np.arange(N_TH, dtype=f) * f(TILE) * psy).astype(f)
    ty1 = (ty0 + f(TILE) * psy).astype(f)
    dxt = np.maximum(np.maximum(tx0[None, :] - mx[:, None], mx[:, None] - tx1[None, :]), f(0.0)).astype(f)
    dyt = np.maximum(np.maximum(ty0[None, :] - my[:, None], my[:, None] - ty1[None, :]), f(0.0)).astype(f)
    r2 = ((radius * f(TILE_CULL_R)) ** 2).astype(f)
    keep_tile = ((dxt[:, None, :] ** 2 + dyt[:, :, None] ** 2) <= r2[:, None, None]) \
        & maskf[:, None, None]                         # [N, nth, ntw]

    hy0 = (tly + np.arange(N_HH, dtype=f) * f(TILE / 2) * psy).astype(f)
    hy1 = (hy0 + f(TILE / 2) * psy).astype(f)
    dyh = np.maximum(np.maximum(hy0[None, :] - my[:, None], my[:, None] - hy1[None, :]), f(0.0)).astype(f)
    rh2 = ((radius * f(HALF_CULL_R)) ** 2).astype(f)
    keep_half = ((dxt[:, None, :] ** 2 + dyh[:, :, None] ** 2) <= rh2[:, None, None]) \
        & keep_tile[:, np.arange(N_HH) // 2, :]        # [N, nhh, ntw]

    detc = np.maximum(det, f(1e-12))
    ia2 = (-(c11 / detc) / 2).astype(f)
    ib2 = (c01 / detc).astype(f)
    ic2 = (-(c00 / detc) / 2).astype(f)
    loga = np.log(a).astype(f)

    skey = np.where(maskf, depth, f(1e10))
    order = np.argsort(skey, kind="stable")
    keep_s = keep_half[order]                          # depth-ordered

    # LPT binning: 512 half-tiles -> 16 streams of exactly 32
    counts = keep_s.sum(axis=0)                        # [nhh, ntw]
    items = sorted(((int(counts[hr, tc]), hr, tc)
                    for hr in range(N_HH) for tc in range(N_TW)), reverse=True)
    loads = [0 if m % 2 == 0 else PH_OFF for m in range(N_STREAMS)]
    stream_map = [[] for _ in range(N_STREAMS)]
    for k, hr, tc in items:
        cands = [m for m in range(N_STREAMS) if len(stream_map[m]) < HT_CAP]
        m = min(cands, key=lambda m: loads[m])
        stream_map[m].append((hr, tc))
        loads[m] += k + 1
    for m in range(N_STREAMS):
        real = loads[m] - (0 if m % 2 == 0 else PH_OFF)
        assert real <= S2A[m % 2], f"stream {m} load {real} > {S2A[m % 2]}"

    coef_all = np.zeros((N_STREAMS, 6, SP2), f)
    coef_all[:, 0, :] = f(-1e4)        # padding slots: wd = 0
    colm_all = np.zeros((N_STREAMS, SP2, 96), f)

    for st in range(N_STREAMS):
        s = 0
        for tloc, (hr, tc) in enumerate(stream_map[st]):
            idx = order[keep_s[:, hr, tc]]            # depth-ordered members
            k = idx.shape[0]
            cxt = tlx + (f(TILE * tc) + f(8.0)) * psx
            cyt = tly + (f(TILE / 2 * hr) + f(4.0)) * psy
            coef_all[st, :, s] = 0.0
            coef_all[st, 0, s] = f(DUMMY_ARG)         # dummy reset slot
            if k:
                mxp = (mx[idx] - cxt).astype(f)
                myp = (my[idx] - cyt).astype(f)
                A, B, Cc = ia2[idx], ib2[idx], ic2[idx]
                sl = slice(s + 1, s + 1 + k)
                coef_all[st, 0, sl] = (A * mxp * mxp + B * mxp * myp + Cc * myp * myp + loga[idx]).astype(f)
                coef_all[st, 1, sl] = (-(2 * A * mxp + B * myp)).astype(f)
                coef_all[st, 2, sl] = (-(2 * Cc * myp + B * mxp)).astype(f)
                coef_all[st, 3, sl] = A
                coef_all[st, 4, sl] = B
                coef_all[st, 5, sl] = Cc
                colm_all[st, sl, 3 * tloc:3 * tloc + 3] = color[idx]
            s += k + 1
        assert s <= S2A[st % 2], f"stream {st} length {s} > {S2A[st % 2]}"

    # fp16 mono with exact half-integer coords (16x8 half-tile, p = i*16+j)
    xs = np.arange(TILE, dtype=f) - f(7.5)
    ys = np.arange(TILE // 2, dtype=f) - f(3.5)
    xg = np.tile(xs, TILE // 2)
    yg = np.repeat(ys, TILE)
    mono6 = np.stack([np.ones_like(xg), xg, yg, xg * xg, xg * yg, yg * yg],
                     0).astype(np.float16)            # [6,128], exact in fp16
    mono12 = np.concatenate([mono6, mono6], axis=0)   # [12,128]
    scale = np.array([1.0, psx, psy, psx * psx, psx * psy, psy * psy], f)
    coef_sc = (coef_all * scale[None, :, None]).astype(f)
    assert np.abs(coef_sc).max() < 6e4
    chi = coef_sc.astype(np.float16)
    clo = (coef_sc - chi.astype(f)).astype(np.float16)
    coef12 = np.concatenate([chi, clo], axis=1)       # [16, 12, SP2]

    cm = np.zeros((N_CORES, 12, 128 + 2 * SP2), np.float16)
    colm = np.zeros((N_CORES, 128, 2 * 3 * 96), np.float16)
    for core in range(N_CORES):
        cm[core, :, 0:128] = mono12
        for ph in range(2):
            st = core * 2 + ph
            cm[core, :, 128 + SP2 * ph:128 + SP2 * (ph + 1)] = coef12[st]
            blk = colm_all[st].reshape(3, 128, 96).transpose(1, 0, 2).reshape(128, 288)
            colm[core, :, 288 * ph:288 * (ph + 1)] = blk.astype(np.float16)
    return cm, colm, stream_map


_COMPILED = None


def _build_program():
    import concourse.bass as bass
    import concourse.tile as tile
    import concourse.mybir as mybir
    from concourse import bacc
    from concourse.masks import make_identity

    # Skip the Bass preamble's all-engine entry barrier (~0.64us): the const
    # APs it fences are written and read on the same Pool queue (FIFO-ordered)
    # and all cross-engine deps in the kernel body are tile-tracked.
    orig_aeb = bass.Bass.all_engine_barrier
    bass.Bass.all_engine_barrier = lambda self, **kw: None
    try:
        nc = bacc.Bacc("TRN2", target_bir_lowering=False, debug=False,
                       num_devices=N_CORES)
    finally:
        bass.Bass.all_engine_barrier = orig_aeb
    dt = mybir.dt.float32
    dth = mybir.dt.float16
    cm_d = nc.dram_tensor("cm", [12, 128 + 2 * SP2], dth, kind="ExternalInput").ap()
    colm_d = nc.dram_tensor("colm", [128, 2 * 288], dth, kind="ExternalInput").ap()
    out_d = nc.dram_tensor("out", [128, 2 * 96], dth, kind="ExternalOutput").ap()

    NCH = SP2 // 128
    Alu = mybir.AluOpType
    Act = mybir.ActivationFunctionType

    # Skip the kernel-tail drain + double all-engine barrier (~0.5us): the
    # runtime re-initializes semaphores at each NEFF launch, so repeat
    # executions stay correct (verified by back-to-back runs).
    orig_drain = tile.TileContext._drain_and_barrier

    def _nodrain(self, tick_clock, wait_clock):
        popped = self.nc._tile_sem_poison_stack.pop()
        assert popped is self._sem_poison

    tile.TileContext._drain_and_barrier = _nodrain
    try:
        with tile.TileContext(nc) as tc:
            with tc.tile_pool(name="cst", bufs=1) as cst, \
                 tc.tile_pool(name="sb", bufs=1) as sb, \
                 tc.tile_pool(name="ps", bufs=1, space="PSUM") as ps, \
                 tc.tile_pool(name="pst", bufs=4, space="PSUM") as pst, \
                 tc.tile_pool(name="pso", bufs=1, space="PSUM") as pso:
                nbias = cst.tile([128, 1], dt)
                nc.gpsimd.memset(nbias[:], 0.0)
                warm = cst.tile([128, 1], dt)
                nc.scalar.activation(warm[:], nbias[:], Act.Exp)  # preload table
                ident = cst.tile([128, 128], dth)
                make_identity(nc, ident[:])
                cm_s = cst.tile([12, 128 + 2 * SP2], dth)
                nc.sync.dma_start(cm_s[:], cm_d[:])
                mono_s = cm_s[:, 0:128]
                colm_s = cst.tile([128, 2 * 288], dth)
                nc.scalar.dma_start(colm_s[:], colm_d[:])
                osb = cst.tile([128, 2 * 96], dth)

                # compute-op chunks over [0, S2A[ph]): big head, short terminal
                cbs = {0: [0, 256, S2A[0]], 1: [0, 224, S2A[1]]}
                arg_ps, wd, om, scan, wt, wtT, oacc = ({} for _ in range(7))
                for ph in range(2):
                    sa = S2A[ph]
                    arg_ps[ph] = ps.tile([128, sa], dt, tag=f"arg{ph}", name=f"arg{ph}")
                    wd[ph] = sb.tile([128, sa], dth, tag=f"wd{ph}", name=f"wd{ph}")
                    om[ph] = sb.tile([128, sa], dth, tag=f"om{ph}", name=f"om{ph}")
                    scan[ph] = sb.tile([128, sa + 1], dth, tag=f"scan{ph}", name=f"scan{ph}")
                    nc.gpsimd.memset(scan[ph][:, 0:1], 16384.0)
                    wt[ph] = sb.tile([128, SP2], dth, tag=f"wt{ph}", name=f"wt{ph}")
                    nc.gpsimd.memset(wt[ph][:, sa:SP2], 0.0)
                    wtT[ph] = sb.tile([128, NCH * 128], dth, tag=f"wtT{ph}", name=f"wtT{ph}")
                    oacc[ph] = pso.tile([128, 96], dt, tag=f"oacc{ph}", name=f"oacc{ph}")

                for ph in range(2):
                    cb = cbs[ph]
                    coef_ph = cm_s[:, 128 + SP2 * ph:128 + SP2 * ph + S2A[ph]]
                    for c in range(len(cb) - 1):
                        lo, hi = cb[c], cb[c + 1]
                        sl = slice(lo, hi)
                        nc.tensor.matmul(arg_ps[ph][:, sl], mono_s,
                                         coef_ph[:, sl], start=True, stop=True)
                        nc.scalar.activation(wd[ph][:, sl], arg_ps[ph][:, sl],
                                             Act.Exp, bias=nbias[:])
                        nc.gpsimd.tensor_scalar(om[ph][:, sl], wd[ph][:, sl],
                                                -1.0, 1.0, Alu.mult, Alu.add)
                        init = 0.0 if c == 0 else scan[ph][:, lo:lo + 1]
                        nc.vector.tensor_tensor_scan(scan[ph][:, lo + 1:hi + 1],
                                                     om[ph][:, sl], wd[ph][:, sl],
                                                     init, Alu.mult, Alu.max)
                        # ph0 diffs on Pool so DVE reaches ph1's scans sooner
                        df_eng = nc.gpsimd if ph == 0 else nc.vector
                        df_eng.tensor_tensor(wt[ph][:, sl], scan[ph][:, lo:hi],
                                             scan[ph][:, lo + 1:hi + 1],
                                             Alu.subtract)
                # back-halves after both phases' compute chains so scans/diffs
                # beat copies into the engine queues
                cp_eng = {(0, 0): nc.scalar, (0, 1): nc.vector, (0, 2): nc.vector,
                          (1, 0): nc.scalar, (1, 1): nc.scalar, (1, 2): nc.vector}
                for ph in range(2):
                    for j in range(NCH):
                        jsl = slice(128 * j, 128 * (j + 1))
                        wtT_ps = pst.tile([128, 128], dth, tag="wtT_ps")
                        nc.tensor.transpose(wtT_ps[:], wt[ph][:, jsl], ident[:])
                        dst = wtT[ph][:, jsl]
                        eng = cp_eng[(ph, j)]
                        if eng is nc.scalar:
                            nc.scalar.activation(dst, wtT_ps[:], Act.Copy)
                        else:
                            eng.tensor_copy(dst, wtT_ps[:])
                        nc.tensor.matmul(oacc[ph][:], wtT[ph][:, jsl],
                                         colm_s[:, 288 * ph + 96 * j:288 * ph + 96 * (j + 1)],
                                         start=(j == 0), stop=(j == NCH - 1))
                    nc.vector.tensor_copy(osb[:, 96 * ph:96 * (ph + 1)], oacc[ph][:])
                # ph1's store on the idle SP queue; ph0's on ACT (emitted last
                # so its wait tails ACT's queue) so its HWDGE stage cannot
                # delay the terminal store
                nc.scalar.dma_start(out_d[:, 0:96], osb[:, 0:96])
                nc.sync.dma_start(out_d[:, 96:192], osb[:, 96:192])
    finally:
        tile.TileContext._drain_and_barrier = orig_drain
    nc.compile()
    return nc


def _get_compiled():
    global _COMPILED
    if _COMPILED is None:
        _COMPILED = _build_program()
    return _COMPILED


def _unshard(results, stream_map):
    out = np.empty((H, W, 3), np.float32)
    for core in range(N_CORES):
        # device accumulates 2^14-prescaled weights; unscale here
        r = np.asarray(results[core]["out"], np.float32) * np.float32(2.0 ** -14)
        for ph in range(2):
            blk = r[:, 96 * ph:96 * (ph + 1)].reshape(8, 16, 32, 3)
            for tloc, (hr, tc) in enumerate(stream_map[core * 2 + ph]):
                out[8 * hr:8 * (hr + 1), 16 * tc:16 * (tc + 1)] = blk[:, :, tloc]
    return out


def run(inputs, trace=False, trace_kwargs=None):
    from concourse.bass_utils import run_bass_kernel_spmd

    cm, colm, stream_map = _host_precompute(**inputs)
    nc = _get_compiled()
    in_maps = [{"cm": np.ascontiguousarray(cm[c]),
                "colm": np.ascontiguousarray(colm[c])} for c in range(N_CORES)]
    res = run_bass_kernel_spmd(nc, in_maps, list(range(N_CORES)),
                               trace=trace, **(trace_kwargs or {}))
    return _unshard(res.results, stream_map), res


def kernel(**inputs) -> np.ndarray:
    out, _ = run(inputs, trace=False)
    return out


# revision 28
# speedup vs baseline: 1.0481x; 1.0481x over previous
"""Half-tile-binned gaussian-splat compositing kernel for 8 TRN2 NeuronCores.

Strategy v2 (histogram binning, fp16 pipeline):
  Host (numpy, exact f32 replication of the reference's per-gaussian math):
    - project gaussians, build inverse 2x2 covs, frustum mask, per-tile
      bounding-circle mask (reference semantics), global depth sort.
    - split each 16x16 tile into top/bottom 16x8 half-tiles; a gaussian is
      kept for a half-tile iff it passes the reference's tile-level test AND
      its bcircle (radius 4r > reference's 3r, so dropped tails have
      G <= e^-8) touches the half-tile box. 512 half-tiles are LPT-balanced
      into 16 streams (8 cores x 2 phases, exactly 32 half-tiles each,
      max ~340 slots incl. one dummy reset slot per half-tile).
    - per slot: 6 quadratic-form coefficients in half-tile-local coords such
      that arg = coef . [1,X,Y,X^2,XY,Y^2] = -0.5*quad + log(alpha); coef is
      split hi+lo fp16 and stacked as 12 rows so ONE K=12 fp16 matmul per
      chunk computes hi+lo exactly; colors are packed [128, 3, 96] fp16 so
      every DMA is fully contiguous (no strided descriptors).
  Device (per core, SPMD over 8; partition dim = 128 pixels of one 16x8
  half-tile, free dim = slot stream, phase-major, 3 chunks of 128 slots):
    - PE:  arg[pix, slot] = mono12^T @ coef12  (fp16, fp32 PSUM)
    - ACT: wd = exp(arg - CEXP + 14*ln2)   -> fp16, 2^14-prescaled so all
           fp16 values stay in normal range (dummy slots hit exp = 2^14)
    - POOL:om = 1 - (e^C/2^14)*wd          (= 1-w; dummy slots -> -2980)
    - DVE: scan_s = max(om_s * scan_{s-1}, wd_s) = 2^14 * transmittance,
           fp32 internal state, fp16 out; resets to 2^14 at dummy slots
    - DVE: weight = Tprev * (e^C/2^28) * wd  (= T*w; the reference's
           T>1e-4 gate is dropped: post-threshold contributions sum to
           <= 1e-4 absolute, far under tolerance; dummy-slot weights are
           killed by their zero color rows)
    - PE:  transpose weight 128-chunks (fp16 identity), ACT/DVE/POOL copy
           to SBUF fp16, accumulate out[pix, 96] += weightT^T @ colors
  Output [128, 2*96] fp32 stored contiguously per phase (ph0 overlaps ph1
  compute); host rearranges into the [256,256,3] image.
"""

import numpy as np

N = 1024
H = 256
W = 256
TILE = 16
FX = 300.0
FY = 300.0
CX = 128.0
CY = 128.0
NEAR = 0.1
FAR = 100.0
FRUSTUM_R = 1.0
TILE_CULL_R = 3.0
HALF_CULL_R = 4.0            # half-tile bcircle radius (dropped tails: G<=e^-8)
N_TH = H // TILE
N_TW = W // TILE
N_HH = 2 * N_TH              # 32 rows of 16x8 half-tiles
N_CORES = 8
N_STREAMS = 2 * N_CORES
HT_CAP = 32                  # half-tiles per stream (colm width 96 = 3*32)
SP2 = 384                    # padded stream length (3 transpose chunks of 128)
S2A = {0: 376, 1: 332}       # active slot columns per phase (ph1 lighter: it
                             # ends the kernel, ph0's extra work is overlapped)
PH_OFF = 45                  # LPT load bias applied to ph1 streams
DUMMY_ARG = float(np.log(np.float64(16384.0)))  # exp(dummy) = 2^14 (scan reset)

f = np.float32


def _sigmoid(x):
    return (1.0 / (1.0 + np.exp(-x.astype(np.float64)))).astype(f)


def _host_precompute(mean, qvec, log_svec, color, alpha, c2w):
    """Replicates reference per-gaussian math in f32, bins per half-tile.

    Returns (cm [8,12,128+2*SP2] fp16, colm [8,128,2*288] fp16,
             stream_map: list of 16 lists of (hr, tc))."""
    mean = np.asarray(mean, f)
    qvec = np.asarray(qvec, f)
    log_svec = np.asarray(log_svec, f)
    color = np.asarray(color, f)
    alpha = np.asarray(alpha, f)
    c2w = np.asarray(c2w, f)

    svec = np.exp(log_svec).astype(f)
    a = _sigmoid(alpha)
    Rcw = c2w[:, :3]
    t = c2w[:, 3]
    mean_cam = ((mean - t) @ Rcw).astype(f)
    depth = mean_cam[:, 2]
    zc = np.maximum(depth, f(1e-6))
    inv_z = (f(1.0) / zc).astype(f)
    x, y = mean_cam[:, 0], mean_cam[:, 1]
    mx, my = (x * inv_z).astype(f), (y * inv_z).astype(f)

    q = (qvec / np.linalg.norm(qvec, axis=-1, keepdims=True)).astype(f)
    qw, qx, qy, qz = q[:, 0], q[:, 1], q[:, 2], q[:, 3]
    r0 = np.stack([1 - 2 * (qy * qy + qz * qz), 2 * (qx * qy - qw * qz), 2 * (qx * qz + qw * qy)], -1)
    r1 = np.stack([2 * (qx * qy + qw * qz), 1 - 2 * (qx * qx + qz * qz), 2 * (qy * qz - qw * qx)], -1)
    r2m = np.stack([2 * (qx * qz - qw * qy), 2 * (qy * qz + qw * qx), 1 - 2 * (qx * qx + qy * qy)], -1)
    Rq = np.stack([r0, r1, r2m], axis=1).astype(f)
    zero = np.zeros_like(inv_z)
    J = np.stack([np.stack([inv_z, zero, (-x * inv_z * inv_z).astype(f)], -1),
                  np.stack([zero, inv_z, (-y * inv_z * inv_z).astype(f)], -1)], axis=1).astype(f)
    cov3d = np.einsum('nij,nj,nkj->nik', Rq, (svec * svec).astype(f), Rq).astype(f)
    JW = np.einsum('nij,kj->nik', J, Rcw).astype(f)
    cov = np.einsum('nij,njk,nlk->nil', JW, cov3d, JW).astype(f)
    cov = ((cov + np.swapaxes(cov, -1, -2)) / 2.0).astype(f)
    c00, c01, c11 = cov[:, 0, 0], cov[:, 0, 1], cov[:, 1, 1]
    m = ((c00 + c11) / 2.0).astype(f)
    det = (c00 * c11 - c01 * c01).astype(f)
    radius = np.sqrt(m + np.sqrt(np.clip((m * m - det).astype(f), 0.0, None))).astype(f)

    r3d = (f(FRUSTUM_R) * np.max(svec, axis=-1)).astype(f)
    half_w = f(W / 2.0) / f(FX)
    half_h = f(H / 2.0) / f(FY)
    marg = (r3d * inv_z).astype(f)
    maskf = ((depth > f(NEAR)) & (depth < f(FAR)) &
             (np.abs(mx) < half_w + marg) & (np.abs(my) < half_h + marg))

    psx, psy = f(1.0) / f(FX), f(1.0) / f(FY)
    tlx, tly = f(-CX) / f(FX), f(-CY) / f(FY)
    tx0 = (tlx + np.arange(N_TW, dtype=f) * f(TILE) * psx).astype(f)
    tx1 = (tx0 + f(TILE) * psx).astype(f)
    ty0 = (tly + np.arange(N_TH, dtype=f) * f(TILE) * psy).astype(f)
    ty1 = (ty0 + f(TILE) * psy).astype(f)
    dxt = np.maximum(np.maximum(tx0[None, :] - mx[:, None], mx[:, None] - tx1[None, :]), f(0.0)).astype(f)
    dyt = np.maximum(np.maximum(ty0[None, :] - my[:, None], my[:, None] - ty1[None, :]), f(0.0)).astype(f)
    r2 = ((radius * f(TILE_CULL_R)) ** 2).astype(f)
    keep_tile = ((dxt[:, None, :] ** 2 + dyt[:, :, None] ** 2) <= r2[:, None, None]) \
        & maskf[:, None, None]                         # [N, nth, ntw]

    hy0 = (tly + np.arange(N_HH, dtype=f) * f(TILE / 2) * psy).astype(f)
    hy1 = (hy0 + f(TILE / 2) * psy).astype(f)
    dyh = np.maximum(np.maximum(hy0[None, :] - my[:, None], my[:, None] - hy1[None, :]), f(0.0)).astype(f)
    rh2 = ((radius * f(HALF_CULL_R)) ** 2).astype(f)
    keep_half = ((dxt[:, None, :] ** 2 + dyh[:, :, None] ** 2) <= rh2[:, None, None]) \
        & keep_tile[:, np.arange(N_HH) // 2, :]        # [N, nhh, ntw]

    detc = np.maximum(det, f(1e-12))
    ia2 = (-(c11 / detc) / 2).astype(f)
    ib2 = (c01 / detc).astype(f)
    ic2 = (-(c00 / detc) / 2).astype(f)
    loga = np.log(a).astype(f)

    skey = np.where(maskf, depth, f(1e10))
    order = np.argsort(skey, kind="stable")
    keep_s = keep_half[order]                          # depth-ordered

    # LPT binning: 512 half-tiles -> 16 streams of exactly 32
    counts = keep_s.sum(axis=0)                        # [nhh, ntw]
    items = sorted(((int(counts[hr, tc]), hr, tc)
                    for hr in range(N_HH) for tc in range(N_TW)), reverse=True)
    loads = [0 if m % 2 == 0 else PH_OFF for m in range(N_STREAMS)]
    stream_map = [[] for _ in range(N_STREAMS)]
    for k, hr, tc in items:
        cands = [m for m in range(N_STREAMS) if len(stream_map[m]) < HT_CAP]
        m = min(cands, key=lambda m: loads[m])
        stream_map[m].append((hr, tc))
        loads[m] += k + 1
    for m in range(N_STREAMS):
        real = loads[m] - (0 if m % 2 == 0 else PH_OFF)
        assert real <= S2A[m % 2], f"stream {m} load {real} > {S2A[m % 2]}"

    coef_all = np.zeros((N_STREAMS, 6, SP2), f)
    coef_all[:, 0, :] = f(-1e4)        # padding slots: wd = 0
    colm_all = np.zeros((N_STREAMS, SP2, 96), f)

    for st in range(N_STREAMS):
        s = 0
        for tloc, (hr, tc) in enumerate(stream_map[st]):
            idx = order[keep_s[:, hr, tc]]            # depth-ordered members
            k = idx.shape[0]
            cxt = tlx + (f(TILE * tc) + f(8.0)) * psx
            cyt = tly + (f(TILE / 2 * hr) + f(4.0)) * psy
            coef_all[st, :, s] = 0.0
            coef_all[st, 0, s] = f(DUMMY_ARG)         # dummy reset slot
            if k:
                mxp = (mx[idx] - cxt).astype(f)
                myp = (my[idx] - cyt).astype(f)
                A, B, Cc = ia2[idx], ib2[idx], ic2[idx]
                sl = slice(s + 1, s + 1 + k)
                coef_all[st, 0, sl] = (A * mxp * mxp + B * mxp * myp + Cc * myp * myp + loga[idx]).astype(f)
                coef_all[st, 1, sl] = (-(2 * A * mxp + B * myp)).astype(f)
                coef_all[st, 2, sl] = (-(2 * Cc * myp + B * mxp)).astype(f)
                coef_all[st, 3, sl] = A
                coef_all[st, 4, sl] = B
                coef_all[st, 5, sl] = Cc
                colm_all[st, sl, 3 * tloc:3 * tloc + 3] = color[idx]
            s += k + 1
        assert s <= S2A[st % 2], f"stream {st} length {s} > {S2A[st % 2]}"

    # fp16 mono with exact half-integer coords (16x8 half-tile, p = i*16+j)
    xs = np.arange(TILE, dtype=f) - f(7.5)
    ys = np.arange(TILE // 2, dtype=f) - f(3.5)
    xg = np.tile(xs, TILE // 2)
    yg = np.repeat(ys, TILE)
    mono6 = np.stack([np.ones_like(xg), xg, yg, xg * xg, xg * yg, yg * yg],
                     0).astype(np.float16)            # [6,128], exact in fp16
    mono12 = np.concatenate([mono6, mono6], axis=0)   # [12,128]
    scale = np.array([1.0, psx, psy, psx * psx, psx * psy, psy * psy], f)
    coef_sc = (coef_all * scale[None, :, None]).astype(f)
    assert np.abs(coef_sc).max() < 6e4
    chi = coef_sc.astype(np.float16)
    clo = (coef_sc - chi.astype(f)).astype(np.float16)
    coef12 = np.concatenate([chi, clo], axis=1)       # [16, 12, SP2]

    cm = np.zeros((N_CORES, 12, 128 + 2 * SP2), np.float16)
    colm = np.zeros((N_CORES, 128, 2 * 3 * 96), np.float16)
    for core in range(N_CORES):
        cm[core, :, 0:128] = mono12
        for ph in range(2):
            st = core * 2 + ph
            cm[core, :, 128 + SP2 * ph:128 + SP2 * (ph + 1)] = coef12[st]
            blk = colm_all[st].reshape(3, 128, 96).transpose(1, 0, 2).reshape(128, 288)
            colm[core, :, 288 * ph:288 * (ph + 1)] = blk.astype(np.float16)
    return cm, colm, stream_map


_COMPILED = None


def _build_program():
    import concourse.bass as bass
    import concourse.tile as tile
    import concourse.mybir as mybir
    from concourse import bacc
    from concourse.masks import make_identity

    # Skip the Bass preamble's all-engine entry barrier (~0.64us): the const
    # APs it fences are written and read on the same Pool queue (FIFO-ordered)
    # and all cross-engine deps in the kernel body are tile-tracked.
    orig_aeb = bass.Bass.all_engine_barrier
    bass.Bass.all_engine_barrier = lambda self, **kw: None
    try:
        nc = bacc.Bacc("TRN2", target_bir_lowering=False, debug=False,
                       num_devices=N_CORES)
    finally:
        bass.Bass.all_engine_barrier = orig_aeb
    dt = mybir.dt.float32
    dth = mybir.dt.float16
    cm_d = nc.dram_tensor("cm", [12, 128 + 2 * SP2], dth, kind="ExternalInput").ap()
    colm_d = nc.dram_tensor("colm", [128, 2 * 288], dth, kind="ExternalInput").ap()
    out_d = nc.dram_tensor("out", [128, 2 * 96], dth, kind="ExternalOutput").ap()

    NCH = SP2 // 128
    Alu = mybir.AluOpType
    Act = mybir.ActivationFunctionType

    # Skip the kernel-tail drain + double all-engine barrier (~0.5us): the
    # runtime re-initializes semaphores at each NEFF launch, so repeat
    # executions stay correct (verified by back-to-back runs).
    orig_drain = tile.TileContext._drain_and_barrier

    def _nodrain(self, tick_clock, wait_clock):
        popped = self.nc._tile_sem_poison_stack.pop()
        assert popped is self._sem_poison

    tile.TileContext._drain_and_barrier = _nodrain
    try:
        with tile.TileContext(nc) as tc:
            with tc.tile_pool(name="cst", bufs=1) as cst, \
                 tc.tile_pool(name="sb", bufs=1) as sb, \
                 tc.tile_pool(name="ps", bufs=1, space="PSUM") as ps, \
                 tc.tile_pool(name="pst", bufs=4, space="PSUM") as pst, \
                 tc.tile_pool(name="pso", bufs=1, space="PSUM") as pso:
                nbias = cst.tile([128, 1], dt)
                nc.gpsimd.memset(nbias[:], 0.0)
                warm = cst.tile([128, 1], dt)
                nc.scalar.activation(warm[:], nbias[:], Act.Exp)  # preload table
                ident = cst.tile([128, 128], dth)
                make_identity(nc, ident[:])
                cm_s = cst.tile([12, 128 + 2 * SP2], dth)
                nc.sync.dma_start(cm_s[:], cm_d[:])
                mono_s = cm_s[:, 0:128]
                colm_s = cst.tile([128, 2 * 288], dth)
                nc.scalar.dma_start(colm_s[:], colm_d[:])
                osb = cst.tile([128, 2 * 96], dth)

                # compute-op chunks over [0, S2A[ph]): big head, short terminal
                cbs = {0: [0, 256, S2A[0]], 1: [0, 224, S2A[1]]}
                arg_ps, wd, om, scan, wt, wtT, oacc = ({} for _ in range(7))
                for ph in range(2):
                    sa = S2A[ph]
                    arg_ps[ph] = ps.tile([128, sa], dt, tag=f"arg{ph}", name=f"arg{ph}")
                    wd[ph] = sb.tile([128, sa], dth, tag=f"wd{ph}", name=f"wd{ph}")
                    om[ph] = sb.tile([128, sa], dth, tag=f"om{ph}", name=f"om{ph}")
                    scan[ph] = sb.tile([128, sa + 1], dth, tag=f"scan{ph}", name=f"scan{ph}")
                    nc.gpsimd.memset(scan[ph][:, 0:1], 16384.0)
                    wt[ph] = sb.tile([128, SP2], dth, tag=f"wt{ph}", name=f"wt{ph}")
                    nc.gpsimd.memset(wt[ph][:, sa:SP2], 0.0)
                    wtT[ph] = sb.tile([128, NCH * 128], dth, tag=f"wtT{ph}", name=f"wtT{ph}")
                    oacc[ph] = pso.tile([128, 96], dt, tag=f"oacc{ph}", name=f"oacc{ph}")

                for ph in range(2):
                    cb = cbs[ph]
                    coef_ph = cm_s[:, 128 + SP2 * ph:128 + SP2 * ph + S2A[ph]]
                    for c in range(len(cb) - 1):
                        lo, hi = cb[c], cb[c + 1]
                        sl = slice(lo, hi)
                        nc.tensor.matmul(arg_ps[ph][:, sl], mono_s,
                                         coef_ph[:, sl], start=True, stop=True)
                        nc.scalar.activation(wd[ph][:, sl], arg_ps[ph][:, sl],
                                             Act.Exp, bias=nbias[:])
                        nc.gpsimd.tensor_scalar(om[ph][:, sl], wd[ph][:, sl],
                                                -1.0, 1.0, Alu.mult, Alu.add)
                        init = 0.0 if c == 0 else scan[ph][:, lo:lo + 1]
                        nc.vector.tensor_tensor_scan(scan[ph][:, lo + 1:hi + 1],
                                                     om[ph][:, sl], wd[ph][:, sl],
                                                     init, Alu.mult, Alu.max)
                        nc.vector.tensor_tensor(wt[ph][:, sl], scan[ph][:, lo:hi],
                                                scan[ph][:, lo + 1:hi + 1],
                                                Alu.subtract)
                # back-halves after both phases' compute chains so scans/diffs
                # beat copies into the engine queues
                cp_eng = {(0, 0): nc.scalar, (0, 1): nc.vector, (0, 2): nc.vector,
                          (1, 0): nc.scalar, (1, 1): nc.scalar, (1, 2): nc.vector}
                for ph in range(2):
                    for j in range(NCH):
                        jsl = slice(128 * j, 128 * (j + 1))
                        wtT_ps = pst.tile([128, 128], dth, tag="wtT_ps")
                        nc.tensor.transpose(wtT_ps[:], wt[ph][:, jsl], ident[:])
                        dst = wtT[ph][:, jsl]
                        eng = cp_eng[(ph, j)]
                        if eng is nc.scalar:
                            nc.scalar.activation(dst, wtT_ps[:], Act.Copy)
                        else:
                            eng.tensor_copy(dst, wtT_ps[:])
                        nc.tensor.matmul(oacc[ph][:], wtT[ph][:, jsl],
                                         colm_s[:, 288 * ph + 96 * j:288 * ph + 96 * (j + 1)],
                                         start=(j == 0), stop=(j == NCH - 1))
                    nc.vector.tensor_copy(osb[:, 96 * ph:96 * (ph + 1)], oacc[ph][:])
                # ph1's store on the idle SP queue; ph0's on ACT (emitted last
                # so its wait tails ACT's queue) so its HWDGE stage cannot
                # delay the terminal store
                nc.scalar.dma_start(out_d[:, 0:96], osb[:, 0:96])
                nc.sync.dma_start(out_d[:, 96:192], osb[:, 96:192])
    finally:
        tile.TileContext._drain_and_barrier = orig_drain
    nc.compile()
    return nc


def _get_compiled():
    global _COMPILED
    if _COMPILED is None:
        _COMPILED = _build_program()
    return _COMPILED


def _unshard(results, stream_map):
    out = np.empty((H, W, 3), np.float32)
    for core in range(N_CORES):
        # device accumulates 2^14-prescaled weights; unscale here
        r = np.asarray(results[core]["out"], np.float32) * np.float32(2.0 ** -14)
        for ph in range(2):
            blk = r[:, 96 * ph:96 * (ph + 1)].reshape(8, 16, 32, 3)
            for tloc, (hr, tc) in enumerate(stream_map[core * 2 + ph]):
                out[8 * hr:8 * (hr + 1), 16 * tc:16 * (tc + 1)] = blk[:, :, tloc]
    return out


def run(inputs, trace=False, trace_kwargs=None):
    from concourse.bass_utils import run_bass_kernel_spmd

    cm, colm, stream_map = _host_precompute(**inputs)
    nc = _get_compiled()
    in_maps = [{"cm": np.ascontiguousarray(cm[c]),
                "colm": np.ascontiguousarray(colm[c])} for c in range(N_CORES)]
    res = run_bass_kernel_spmd(nc, in_maps, list(range(N_CORES)),
                               trace=trace, **(trace_kwargs or {}))
    return _unshard(res.results, stream_map), res


def kernel(**inputs) -> np.ndarray:
    out, _ = run(inputs, trace=False)
    return out


# revision 31
# speedup vs baseline: 1.0949x; 1.0447x over previous
"""Half-tile-binned gaussian-splat compositing kernel for 8 TRN2 NeuronCores.

Strategy v2 (histogram binning, fp16 pipeline):
  Host (numpy, exact f32 replication of the reference's per-gaussian math):
    - project gaussians, build inverse 2x2 covs, frustum mask, per-tile
      bounding-circle mask (reference semantics), global depth sort.
    - split each 16x16 tile into top/bottom 16x8 half-tiles; a gaussian is
      kept for a half-tile iff it passes the reference's tile-level test AND
      its bcircle (radius 4r > reference's 3r, so dropped tails have
      G <= e^-8) touches the half-tile box. 512 half-tiles are LPT-balanced
      into 16 streams (8 cores x 2 phases, exactly 32 half-tiles each,
      max ~340 slots incl. one dummy reset slot per half-tile).
    - per slot: 6 quadratic-form coefficients in half-tile-local coords such
      that arg = coef . [1,X,Y,X^2,XY,Y^2] = -0.5*quad + log(alpha); coef is
      split hi+lo fp16 and stacked as 12 rows so ONE K=12 fp16 matmul per
      chunk computes hi+lo exactly; colors are packed [128, 3, 96] fp16 so
      every DMA is fully contiguous (no strided descriptors).
  Device (per core, SPMD over 8; partition dim = 128 pixels of one 16x8
  half-tile, free dim = slot stream, phase-major, 3 chunks of 128 slots):
    - PE:  arg[pix, slot] = mono12^T @ coef12  (fp16, fp32 PSUM)
    - ACT: wd = exp(arg - CEXP + 14*ln2)   -> fp16, 2^14-prescaled so all
           fp16 values stay in normal range (dummy slots hit exp = 2^14)
    - POOL:om = 1 - (e^C/2^14)*wd          (= 1-w; dummy slots -> -2980)
    - DVE: scan_s = max(om_s * scan_{s-1}, wd_s) = 2^14 * transmittance,
           fp32 internal state, fp16 out; resets to 2^14 at dummy slots
    - DVE: weight = Tprev * (e^C/2^28) * wd  (= T*w; the reference's
           T>1e-4 gate is dropped: post-threshold contributions sum to
           <= 1e-4 absolute, far under tolerance; dummy-slot weights are
           killed by their zero color rows)
    - PE:  transpose weight 128-chunks (fp16 identity), ACT/DVE/POOL copy
           to SBUF fp16, accumulate out[pix, 96] += weightT^T @ colors
  Output [128, 2*96] fp32 stored contiguously per phase (ph0 overlaps ph1
  compute); host rearranges into the [256,256,3] image.
"""

import numpy as np

N = 1024
H = 256
W = 256
TILE = 16
FX = 300.0
FY = 300.0
CX = 128.0
CY = 128.0
NEAR = 0.1
FAR = 100.0
FRUSTUM_R = 1.0
TILE_CULL_R = 3.0
HALF_CULL_R = 4.0            # half-tile bcircle radius (dropped tails: G<=e^-8)
N_TH = H // TILE
N_TW = W // TILE
N_HH = 2 * N_TH              # 32 rows of 16x8 half-tiles
N_CORES = 8
N_STREAMS = 2 * N_CORES
HT_CAP = 32                  # half-tiles per stream (colm width 96 = 3*32)
SP2 = 384                    # padded stream length (3 transpose chunks of 128)
S2A = {0: 376, 1: 332}       # active slot columns per phase (ph1 lighter: it
                             # ends the kernel, ph0's extra work is overlapped)
PH_OFF = 45                  # LPT load bias applied to ph1 streams
DUMMY_ARG = float(np.log(np.float64(16384.0)))  # exp(dummy) = 2^14 (scan reset)

f = np.float32


def _sigmoid(x):
    return (1.0 / (1.0 + np.exp(-x.astype(np.float64)))).astype(f)


def _host_precompute(mean, qvec, log_svec, color, alpha, c2w):
    """Replicates reference per-gaussian math in f32, bins per half-tile.

    Returns (cm [8,12,128+2*SP2] fp16, colm [8,128,2*288] fp16,
             stream_map: list of 16 lists of (hr, tc))."""
    mean = np.asarray(mean, f)
    qvec = np.asarray(qvec, f)
    log_svec = np.asarray(log_svec, f)
    color = np.asarray(color, f)
    alpha = np.asarray(alpha, f)
    c2w = np.asarray(c2w, f)

    svec = np.exp(log_svec).astype(f)
    a = _sigmoid(alpha)
    Rcw = c2w[:, :3]
    t = c2w[:, 3]
    mean_cam = ((mean - t) @ Rcw).astype(f)
    depth = mean_cam[:, 2]
    zc = np.maximum(depth, f(1e-6))
    inv_z = (f(1.0) / zc).astype(f)
    x, y = mean_cam[:, 0], mean_cam[:, 1]
    mx, my = (x * inv_z).astype(f), (y * inv_z).astype(f)

    q = (qvec / np.linalg.norm(qvec, axis=-1, keepdims=True)).astype(f)
    qw, qx, qy, qz = q[:, 0], q[:, 1], q[:, 2], q[:, 3]
    r0 = np.stack([1 - 2 * (qy * qy + qz * qz), 2 * (qx * qy - qw * qz), 2 * (qx * qz + qw * qy)], -1)
    r1 = np.stack([2 * (qx * qy + qw * qz), 1 - 2 * (qx * qx + qz * qz), 2 * (qy * qz - qw * qx)], -1)
    r2m = np.stack([2 * (qx * qz - qw * qy), 2 * (qy * qz + qw * qx), 1 - 2 * (qx * qx + qy * qy)], -1)
    Rq = np.stack([r0, r1, r2m], axis=1).astype(f)
    zero = np.zeros_like(inv_z)
    J = np.stack([np.stack([inv_z, zero, (-x * inv_z * inv_z).astype(f)], -1),
                  np.stack([zero, inv_z, (-y * inv_z * inv_z).astype(f)], -1)], axis=1).astype(f)
    cov3d = np.einsum('nij,nj,nkj->nik', Rq, (svec * svec).astype(f), Rq).astype(f)
    JW = np.einsum('nij,kj->nik', J, Rcw).astype(f)
    cov = np.einsum('nij,njk,nlk->nil', JW, cov3d, JW).astype(f)
    cov = ((cov + np.swapaxes(cov, -1, -2)) / 2.0).astype(f)
    c00, c01, c11 = cov[:, 0, 0], cov[:, 0, 1], cov[:, 1, 1]
    m = ((c00 + c11) / 2.0).astype(f)
    det = (c00 * c11 - c01 * c01).astype(f)
    radius = np.sqrt(m + np.sqrt(np.clip((m * m - det).astype(f), 0.0, None))).astype(f)

    r3d = (f(FRUSTUM_R) * np.max(svec, axis=-1)).astype(f)
    half_w = f(W / 2.0) / f(FX)
    half_h = f(H / 2.0) / f(FY)
    marg = (r3d * inv_z).astype(f)
    maskf = ((depth > f(NEAR)) & (depth < f(FAR)) &
             (np.abs(mx) < half_w + marg) & (np.abs(my) < half_h + marg))

    psx, psy = f(1.0) / f(FX), f(1.0) / f(FY)
    tlx, tly = f(-CX) / f(FX), f(-CY) / f(FY)
    tx0 = (tlx + np.arange(N_TW, dtype=f) * f(TILE) * psx).astype(f)
    tx1 = (tx0 + f(TILE) * psx).astype(f)
    ty0 = (tly + np.arange(N_TH, dtype=f) * f(TILE) * psy).astype(f)
    ty1 = (ty0 + f(TILE) * psy).astype(f)
    dxt = np.maximum(np.maximum(tx0[None, :] - mx[:, None], mx[:, None] - tx1[None, :]), f(0.0)).astype(f)
    dyt = np.maximum(np.maximum(ty0[None, :] - my[:, None], my[:, None] - ty1[None, :]), f(0.0)).astype(f)
    r2 = ((radius * f(TILE_CULL_R)) ** 2).astype(f)
    keep_tile = ((dxt[:, None, :] ** 2 + dyt[:, :, None] ** 2) <= r2[:, None, None]) \
        & maskf[:, None, None]                         # [N, nth, ntw]

    hy0 = (tly + np.arange(N_HH, dtype=f) * f(TILE / 2) * psy).astype(f)
    hy1 = (hy0 + f(TILE / 2) * psy).astype(f)
    dyh = np.maximum(np.maximum(hy0[None, :] - my[:, None], my[:, None] - hy1[None, :]), f(0.0)).astype(f)
    rh2 = ((radius * f(HALF_CULL_R)) ** 2).astype(f)
    keep_half = ((dxt[:, None, :] ** 2 + dyh[:, :, None] ** 2) <= rh2[:, None, None]) \
        & keep_tile[:, np.arange(N_HH) // 2, :]        # [N, nhh, ntw]

    detc = np.maximum(det, f(1e-12))
    ia2 = (-(c11 / detc) / 2).astype(f)
    ib2 = (c01 / detc).astype(f)
    ic2 = (-(c00 / detc) / 2).astype(f)
    loga = np.log(a).astype(f)

    skey = np.where(maskf, depth, f(1e10))
    order = np.argsort(skey, kind="stable")
    keep_s = keep_half[order]                          # depth-ordered

    # LPT binning: 512 half-tiles -> 16 streams of exactly 32
    counts = keep_s.sum(axis=0)                        # [nhh, ntw]
    items = sorted(((int(counts[hr, tc]), hr, tc)
                    for hr in range(N_HH) for tc in range(N_TW)), reverse=True)
    loads = [0 if m % 2 == 0 else PH_OFF for m in range(N_STREAMS)]
    stream_map = [[] for _ in range(N_STREAMS)]
    for k, hr, tc in items:
        cands = [m for m in range(N_STREAMS) if len(stream_map[m]) < HT_CAP]
        m = min(cands, key=lambda m: loads[m])
        stream_map[m].append((hr, tc))
        loads[m] += k + 1
    for m in range(N_STREAMS):
        real = loads[m] - (0 if m % 2 == 0 else PH_OFF)
        assert real <= S2A[m % 2], f"stream {m} load {real} > {S2A[m % 2]}"

    coef_all = np.zeros((N_STREAMS, 6, SP2), f)
    coef_all[:, 0, :] = f(-1e4)        # padding slots: wd = 0
    colm_all = np.zeros((N_STREAMS, SP2, 96), f)

    for st in range(N_STREAMS):
        s = 0
        for tloc, (hr, tc) in enumerate(stream_map[st]):
            idx = order[keep_s[:, hr, tc]]            # depth-ordered members
            k = idx.shape[0]
            cxt = tlx + (f(TILE * tc) + f(8.0)) * psx
            cyt = tly + (f(TILE / 2 * hr) + f(4.0)) * psy
            coef_all[st, :, s] = 0.0
            coef_all[st, 0, s] = f(DUMMY_ARG)         # dummy reset slot
            if k:
                mxp = (mx[idx] - cxt).astype(f)
                myp = (my[idx] - cyt).astype(f)
                A, B, Cc = ia2[idx], ib2[idx], ic2[idx]
                sl = slice(s + 1, s + 1 + k)
                coef_all[st, 0, sl] = (A * mxp * mxp + B * mxp * myp + Cc * myp * myp + loga[idx]).astype(f)
                coef_all[st, 1, sl] = (-(2 * A * mxp + B * myp)).astype(f)
                coef_all[st, 2, sl] = (-(2 * Cc * myp + B * mxp)).astype(f)
                coef_all[st, 3, sl] = A
                coef_all[st, 4, sl] = B
                coef_all[st, 5, sl] = Cc
                colm_all[st, sl, 3 * tloc:3 * tloc + 3] = color[idx]
            s += k + 1
        assert s <= S2A[st % 2], f"stream {st} length {s} > {S2A[st % 2]}"

    # fp16 mono with exact half-integer coords (16x8 half-tile, p = i*16+j)
    xs = np.arange(TILE, dtype=f) - f(7.5)
    ys = np.arange(TILE // 2, dtype=f) - f(3.5)
    xg = np.tile(xs, TILE // 2)
    yg = np.repeat(ys, TILE)
    mono6 = np.stack([np.ones_like(xg), xg, yg, xg * xg, xg * yg, yg * yg],
                     0).astype(np.float16)            # [6,128], exact in fp16
    mono12 = np.concatenate([mono6, mono6], axis=0)   # [12,128]
    scale = np.array([1.0, psx, psy, psx * psx, psx * psy, psy * psy], f)
    coef_sc = (coef_all * scale[None, :, None]).astype(f)
    assert np.abs(coef_sc).max() < 6e4
    chi = coef_sc.astype(np.float16)
    clo = (coef_sc - chi.astype(f)).astype(np.float16)
    coef12 = np.concatenate([chi, clo], axis=1)       # [16, 12, SP2]

    # telescoped color matmul: out = sum_k scan[k] * D[k], D[k] = C[k]-C[k-1]
    dcol = np.zeros((N_STREAMS, SP2, 96), f)
    dcol[:, 0, :] = colm_all[:, 0, :]
    dcol[:, 1:, :] = colm_all[:, 1:, :] - colm_all[:, :-1, :]

    cm = np.zeros((N_CORES, 12, 128 + 2 * SP2), np.float16)
    colm = np.zeros((N_CORES, 128, 2 * 3 * 96), np.float16)
    for core in range(N_CORES):
        cm[core, :, 0:128] = mono12
        for ph in range(2):
            st = core * 2 + ph
            cm[core, :, 128 + SP2 * ph:128 + SP2 * (ph + 1)] = coef12[st]
            blk = dcol[st].reshape(3, 128, 96).transpose(1, 0, 2).reshape(128, 288)
            colm[core, :, 288 * ph:288 * (ph + 1)] = blk.astype(np.float16)
    return cm, colm, stream_map


_COMPILED = None


def _build_program():
    import concourse.bass as bass
    import concourse.tile as tile
    import concourse.mybir as mybir
    from concourse import bacc
    from concourse.masks import make_identity

    # Skip the Bass preamble's all-engine entry barrier (~0.64us): the const
    # APs it fences are written and read on the same Pool queue (FIFO-ordered)
    # and all cross-engine deps in the kernel body are tile-tracked.
    orig_aeb = bass.Bass.all_engine_barrier
    bass.Bass.all_engine_barrier = lambda self, **kw: None
    try:
        nc = bacc.Bacc("TRN2", target_bir_lowering=False, debug=False,
                       num_devices=N_CORES)
    finally:
        bass.Bass.all_engine_barrier = orig_aeb
    dt = mybir.dt.float32
    dth = mybir.dt.float16
    cm_d = nc.dram_tensor("cm", [12, 128 + 2 * SP2], dth, kind="ExternalInput").ap()
    colm_d = nc.dram_tensor("colm", [128, 2 * 288], dth, kind="ExternalInput").ap()
    out_d = nc.dram_tensor("out", [128, 2 * 96], dth, kind="ExternalOutput").ap()

    NCH = SP2 // 128
    Alu = mybir.AluOpType
    Act = mybir.ActivationFunctionType

    # Skip the kernel-tail drain + double all-engine barrier (~0.5us): the
    # runtime re-initializes semaphores at each NEFF launch, so repeat
    # executions stay correct (verified by back-to-back runs).
    orig_drain = tile.TileContext._drain_and_barrier

    def _nodrain(self, tick_clock, wait_clock):
        popped = self.nc._tile_sem_poison_stack.pop()
        assert popped is self._sem_poison

    tile.TileContext._drain_and_barrier = _nodrain
    try:
        with tile.TileContext(nc) as tc:
            with tc.tile_pool(name="cst", bufs=1) as cst, \
                 tc.tile_pool(name="sb", bufs=1) as sb, \
                 tc.tile_pool(name="ps", bufs=1, space="PSUM") as ps, \
                 tc.tile_pool(name="pst", bufs=4, space="PSUM") as pst, \
                 tc.tile_pool(name="pso", bufs=1, space="PSUM") as pso:
                nbias = cst.tile([128, 1], dt)
                nc.gpsimd.memset(nbias[:], 0.0)
                warm = cst.tile([128, 1], dt)
                nc.scalar.activation(warm[:], nbias[:], Act.Exp)  # preload table
                ident = cst.tile([128, 128], dth)
                make_identity(nc, ident[:])
                cm_s = cst.tile([12, 128 + 2 * SP2], dth)
                nc.sync.dma_start(cm_s[:], cm_d[:])
                mono_s = cm_s[:, 0:128]
                colm_s = cst.tile([128, 2 * 288], dth)
                nc.scalar.dma_start(colm_s[:], colm_d[:])
                osb = cst.tile([128, 2 * 96], dth)

                # compute-op chunks over [0, S2A[ph]): big head, short terminal
                cbs = {0: [0, 256, S2A[0]], 1: [0, 224, S2A[1]]}
                arg_ps, wd, om, scan, wt, wtT, oacc = ({} for _ in range(7))
                for ph in range(2):
                    sa = S2A[ph]
                    arg_ps[ph] = ps.tile([128, sa], dt, tag=f"arg{ph}", name=f"arg{ph}")
                    wd[ph] = sb.tile([128, sa], dth, tag=f"wd{ph}", name=f"wd{ph}")
                    om[ph] = sb.tile([128, sa], dth, tag=f"om{ph}", name=f"om{ph}")
                    scan[ph] = sb.tile([128, SP2], dth, tag=f"scan{ph}", name=f"scan{ph}")
                    nc.gpsimd.memset(scan[ph][:, 0:1], 0.0)
                    nc.gpsimd.memset(scan[ph][:, sa + 1:SP2], 0.0)
                    wtT[ph] = sb.tile([128, NCH * 128], dth, tag=f"wtT{ph}", name=f"wtT{ph}")
                    oacc[ph] = pso.tile([128, 96], dt, tag=f"oacc{ph}", name=f"oacc{ph}")

                for ph in range(2):
                    cb = cbs[ph]
                    coef_ph = cm_s[:, 128 + SP2 * ph:128 + SP2 * ph + S2A[ph]]
                    for c in range(len(cb) - 1):
                        lo, hi = cb[c], cb[c + 1]
                        sl = slice(lo, hi)
                        nc.tensor.matmul(arg_ps[ph][:, sl], mono_s,
                                         coef_ph[:, sl], start=True, stop=True)
                        nc.scalar.activation(wd[ph][:, sl], arg_ps[ph][:, sl],
                                             Act.Exp, bias=nbias[:])
                        nc.gpsimd.tensor_scalar(om[ph][:, sl], wd[ph][:, sl],
                                                -1.0, 1.0, Alu.mult, Alu.add)
                        init = 0.0 if c == 0 else scan[ph][:, lo:lo + 1]
                        nc.vector.tensor_tensor_scan(scan[ph][:, lo + 1:hi + 1],
                                                     om[ph][:, sl], wd[ph][:, sl],
                                                     init, Alu.mult, Alu.max)
                # back-halves after both phases' compute chains so scans beat
                # copies into the engine queues; the color matmul consumes the
                # scan directly (telescoped against host-differenced colors)
                cp_eng = {(0, 0): nc.scalar, (0, 1): nc.vector, (0, 2): nc.vector,
                          (1, 0): nc.scalar, (1, 1): nc.scalar, (1, 2): nc.vector}
                for ph in range(2):
                    for j in range(NCH):
                        jsl = slice(128 * j, 128 * (j + 1))
                        wtT_ps = pst.tile([128, 128], dth, tag="wtT_ps")
                        nc.tensor.transpose(wtT_ps[:], scan[ph][:, jsl], ident[:])
                        dst = wtT[ph][:, jsl]
                        eng = cp_eng[(ph, j)]
                        if eng is nc.scalar:
                            nc.scalar.activation(dst, wtT_ps[:], Act.Copy)
                        else:
                            eng.tensor_copy(dst, wtT_ps[:])
                        nc.tensor.matmul(oacc[ph][:], wtT[ph][:, jsl],
                                         colm_s[:, 288 * ph + 96 * j:288 * ph + 96 * (j + 1)],
                                         start=(j == 0), stop=(j == NCH - 1))
                    ob_eng = nc.vector if ph == 0 else nc.scalar
                    if ob_eng is nc.scalar:
                        nc.scalar.activation(osb[:, 96 * ph:96 * (ph + 1)],
                                             oacc[ph][:], Act.Copy)
                    else:
                        ob_eng.tensor_copy(osb[:, 96 * ph:96 * (ph + 1)], oacc[ph][:])
                # single merged store: one HWDGE occupancy on the idle SP queue
                nc.sync.dma_start(out_d[:], osb[:])
    finally:
        tile.TileContext._drain_and_barrier = orig_drain
    nc.compile()
    return nc


def _get_compiled():
    global _COMPILED
    if _COMPILED is None:
        _COMPILED = _build_program()
    return _COMPILED


def _unshard(results, stream_map):
    out = np.empty((H, W, 3), np.float32)
    for core in range(N_CORES):
        # device accumulates 2^14-prescaled weights; unscale here
        r = np.asarray(results[core]["out"], np.float32) * np.float32(2.0 ** -14)
        for ph in range(2):
            blk = r[:, 96 * ph:96 * (ph + 1)].reshape(8, 16, 32, 3)
            for tloc, (hr, tc) in enumerate(stream_map[core * 2 + ph]):
                out[8 * hr:8 * (hr + 1), 16 * tc:16 * (tc + 1)] = blk[:, :, tloc]
    return out


def run(inputs, trace=False, trace_kwargs=None):
    from concourse.bass_utils import run_bass_kernel_spmd

    cm, colm, stream_map = _host_precompute(**inputs)
    nc = _get_compiled()
    in_maps = [{"cm": np.ascontiguousarray(cm[c]),
                "colm": np.ascontiguousarray(colm[c])} for c in range(N_CORES)]
    res = run_bass_kernel_spmd(nc, in_maps, list(range(N_CORES)),
                               trace=trace, **(trace_kwargs or {}))
    return _unshard(res.results, stream_map), res


def kernel(**inputs) -> np.ndarray:
    out, _ = run(inputs, trace=False)
    return out
